# revision 87
# baseline (speedup 1.0000x reference)
"""Trainium2 Bass kernel for nn_CIAM patch-attention module.

Shapes (hardcoded): x [8, 64, 256, 256] f32, size=4.
Sharding: pure data parallel - one sample per NeuronCore (8 cores).

Per-core structure: the image is split into TOP/BOTTOM halves (128 rows each)
processed as two pipelined passes. Partition p = image row, free = c*256 + w
(w = wi*4 + b). x is pre-cast to bf16 on host and y is stored bf16 (cast back
on host), so all DMAs are plain HWDGE (no SWDGE descriptor-gen on Pool; the
cost model charges out-side bytes, halving store traffic). Phase 2 (64x64 FC
attention) runs in 2-group chunks: PE transposes + ACT evacs/sigmoid; m_e is
materialized with only 2 physical b-copies - phase 3 reads each value 4x via
a [stride-0,2][stride-1,2] innermost AP pair - which halves the PSUM->SBUF
evacuation and the b-expand matmul width. Phase 3 (p1 = x*m) is quarter-
chunked and split DVE/Pool. Phase 4's channel sum/max trees are w-quarter-
chunked big-view ops over a single p1 tile per half, so each quarter's tree
fires as soon as its phase-3 quarter lands (sum tree in one buffer, max tree
in another - they never alias). Per-patch gates fold via one merged
transpose; the whole gate chain is bf16. Phase 5 (y = p1*G) splits DVE/Pool
and streams stores per 8-channel tile. Pool only ever runs mult/copy ops
(TensorTensor max is not legal on Pool in neuronxcc).
"""
import sys
sys.path.insert(0, "/opt/trn_rl_repo")
import numpy as np

_CACHE = {}

B, C, H, W = 8, 64, 256, 256
S = 4
P = 128                # partitions = rows of one half-image
NV = 2                 # image halves (top/bottom)
HIV = P // S           # 32 patch rows per half
WI = W // S            # 64 patch cols
FPC = W                # free elems per channel (one row)
FH = C * FPC           # 16384 free elems per partition per half
CT = 8                 # channels per load tile
NT = C // CT           # 8 tiles
TF = CT * FPC          # 2048 free elems per (half, tile)


def _build():
    import concourse.bass as bass
    import concourse.bacc as bacc
    import concourse.tile as tile
    from concourse import mybir
    from concourse.masks import make_identity

    f32 = mybir.dt.float32
    bf16 = mybir.dt.bfloat16
    AL = mybir.AluOpType
    AF = mybir.ActivationFunctionType

    nc = bacc.Bacc("TRN2", target_bir_lowering=False, debug=False, num_devices=8)

    x_d = nc.dram_tensor("x", [C, H, W], bf16, kind="ExternalInput")
    fcwT_d = nc.dram_tensor("fcwT", [C, C], bf16, kind="ExternalInput")
    fcb_d = nc.dram_tensor("fcb", [C], f32, kind="ExternalInput")
    cws_d = nc.dram_tensor("cws", [6], f32, kind="ExternalInput")
    emat_d = nc.dram_tensor("emat", [C, 2 * C], bf16, kind="ExternalInput")
    y_d = nc.dram_tensor("y", [C, H, W], bf16, kind="ExternalOutput")

    # DRAM views: [half, row-in-half, c, w]
    x_v = x_d[:].rearrange("c (v r) w -> v r c w", v=NV)
    y_v = y_d[:].rearrange("c (v r) w -> v r c w", v=NV)

    with tile.TileContext(nc) as tc:
        with tc.tile_pool(name="big", bufs=1) as big, \
             tc.tile_pool(name="med", bufs=2) as med, \
             tc.tile_pool(name="sm", bufs=2) as sm, \
             tc.tile_pool(name="consts", bufs=1) as consts, \
             tc.tile_pool(name="ps", bufs=1, space="PSUM") as ps:

            # ---- constants (tiles only; DMAs emitted after the x loads) ----
            fcw = consts.tile([C, C], bf16)
            fcb = consts.tile([C, 1], f32)
            cws = consts.tile([P, 6], f32)
            emat = consts.tile([C, 2 * C], bf16)
            ident = consts.tile([P, P], bf16)
            identf = consts.tile([P, P], f32)

            def emit_consts():
                nc.sync.dma_start(out=fcw, in_=fcwT_d[:])         # pre-cast bf16, HWDGE
                nc.sync.dma_start(out=fcb, in_=fcb_d[:].unsqueeze(1))
                nc.sync.dma_start(out=cws, in_=bass.AP(tensor=cws_d, offset=0, ap=[[0, P], [1, 6]]))
                nc.sync.dma_start(out=emat, in_=emat_d[:])
                make_identity(nc, ident)
                make_identity(nc, identf)

            def emit_half(v):
                # loads first so HWDGE starts streaming x before anything else
                xbs = []   # (tile, first-ct, n-ct)
                sizes = [1] * NT
                ct0 = 0
                for nct in sizes:
                    xt = big.tile([P, nct * TF], bf16, tag=f"xb{v}", bufs=NT)
                    xbs.append((xt, ct0, nct))
                    nc.sync.dma_start(out=xt.rearrange("p (c w) -> p c w", c=nct * CT),
                                      in_=x_v[v, :, ct0 * CT:(ct0 + nct) * CT, :])
                    ct0 += nct

                yield
                # ---------- Phase 1: max over b (in-row patch pixels) -----------
                chmaxB = med.tile([P, C * WI], bf16, tag="chmax", bufs=2)  # wi-major: wi*64+c
                for ti, (xt, ct0, nct) in enumerate(xbs):
                    eng1 = nc.vector
                    for s_ in range(nct):
                        ct = ct0 + s_
                        v4 = xt[:, s_ * TF:(s_ + 1) * TF].rearrange("p (r pr u) -> p r pr u", pr=2, u=2)
                        r1 = sm.tile([P, CT * WI, 2], bf16, tag="r1", bufs=2)
                        eng1.tensor_tensor(out=r1, in0=v4[:, :, 0, :], in1=v4[:, :, 1, :], op=AL.max)
                        outv = chmaxB.rearrange("p (wi c) -> p c wi", c=C)[:, ct * CT:(ct + 1) * CT, :]
                        eng1.tensor_tensor(out=outv, in0=r1[:, :, 0], in1=r1[:, :, 1], op=AL.max)

                yield
                # ---------- Phase 2: FC attention -> m_e ------------------------
                # per group of 8 wi: build rhs [c, 8*32], one fc matmul (N=256),
                # one batched sigmoid (+a-dup), 8 transpose+b-expand matmuls with
                # the constant E matrix, one batched evacuation into m_e.
                # m_e as 4 wi-quarter tiles [c, wl(16), b] so P3 can start per quarter
                # m_e stores each (c, wi) gate value twice (t=2); phase 3 reads
                # it 4x via a [stride-0, 2][stride-1, 2] innermost AP pair
                m_eqs = []
                for q_ in range(4):
                    m_eq = med.tile([P, C * W // 8], bf16, tag="me", bufs=4)
                    m_eqs.append(m_eq)
                p1big = big.tile([P, FH], bf16, tag="p1", bufs=2)

                GW = 8                       # wi per group
                def emit_p2_chunk(qc):
                  for g in (2 * qc, 2 * qc + 1):
                    # 4 transposed chmax slices into one psum tile, one evac,
                    # one batched a-fold, two fold+scatter ops -> rhs_w
                    pa4 = ps.tile([P, 4 * P], bf16, tag="pa", bufs=2)
                    for j2 in range(4):
                        j = g * 4 + j2
                        nc.tensor.transpose(pa4[:, j2 * P:(j2 + 1) * P],
                                            chmaxB[:, j * P:(j + 1) * P], ident)
                    pae4 = sm.tile([P, 4 * P], bf16, tag="pae", bufs=1)
                    nc.scalar.copy(out=pae4, in_=pa4)
                    pav = pae4.rearrange("q (jj hi a) -> q (jj hi) a", jj=4, a=S)
                    f1 = sm.tile([P, 4 * HIV, 2], bf16, tag="f1", bufs=1)
                    nc.vector.tensor_tensor(out=f1, in0=pav[:, :, 0:2], in1=pav[:, :, 2:4], op=AL.max)
                    rhs_w = sm.tile([C, GW * HIV], bf16, tag="rhs_w", bufs=2)
                    rhs_b = rhs_w.rearrange("c (blk hi) -> c blk hi", hi=HIV)
                    for k in range(2):
                        # block index (2*jj + k) maps to wi = g*8 + block
                        nc.vector.tensor_tensor(
                            out=rhs_b[:, k:GW:2, :],
                            in0=f1[k * C:(k + 1) * C, :, 0].rearrange("c (jj hi) -> c jj hi", jj=4),
                            in1=f1[k * C:(k + 1) * C, :, 1].rearrange("c (jj hi) -> c jj hi", jj=4),
                            op=AL.max)
                    pmw = ps.tile([C, GW * HIV], f32, tag="pmw", bufs=2)
                    nc.tensor.matmul(pmw, fcw, rhs_w, start=True, stop=True)
                    # sigmoid + duplicate each hi column over the 4 patch rows
                    s2w = sm.tile([C, GW * P], bf16, tag="s2w", bufs=1)
                    nc.scalar.activation(
                        out=s2w.rearrange("c (wl hi a) -> c wl hi a", wl=GW, a=S),
                        in_=pmw.rearrange("c (wl hi) -> c wl hi", wl=GW).unsqueeze(3).broadcast_to([C, GW, HIV, S]),
                        func=AF.Sigmoid, bias=fcb, scale=1.0)
                    for sg in range(2):
                        pe4 = ps.tile([P, GW // 2 * C * 2], f32, tag="pe4", bufs=1)
                        for wl2 in range(GW // 2):
                            wl = sg * (GW // 2) + wl2
                            nc.tensor.matmul(pe4[:, wl2 * C * 2:(wl2 + 1) * C * 2],
                                             s2w[:, wl * P:(wl + 1) * P],
                                             emat, start=True, stop=True)
                        # batched evacuation: psum [(wl c t)] -> m_eq (c, wi, t)
                        w0l = (g % 2) * GW + sg * (GW // 2)
                        me_v = m_eqs[g // 2].rearrange("p (c wi t) -> p wi c t", c=C, t=2)[:, w0l:w0l + GW // 2, :, :]
                        nc.scalar.copy(out=me_v, in_=pe4.rearrange("p (wl c t) -> p wl c t", wl=GW // 2, t=2))

                def emit_p3_chunk(q_):
                    # phase 3 for quarter q_: p1 = x * m over all ct tiles;
                    # in1 reads each m value 4x via [0-stride,2][1-stride,2]
                    WQ = W // 4
                    NWI = WQ // S
                    for t3, (xt, ct0, nct) in enumerate(xbs):
                        p1t = p1big[:, t3 * TF:(t3 + 1) * TF]
                        ncc = nct * CT
                        eng3 = nc.gpsimd if t3 in (2, 5, 6) else nc.vector
                        eng3.tensor_tensor(
                            out=p1t.rearrange("p (c wi d t) -> p c wi d t", c=ncc, d=2, t=2)[:, :, q_ * NWI:(q_ + 1) * NWI, :, :],
                            in0=xt.rearrange("p (c wi d t) -> p c wi d t", c=ncc, d=2, t=2)[:, :, q_ * NWI:(q_ + 1) * NWI, :, :],
                            in1=m_eqs[q_].rearrange("p (c wi t) -> p c wi t", c=C, t=2)[:, ct0 * CT:(ct0 + nct) * CT, :, :]
                                .unsqueeze(3).broadcast_to([P, ncc, NWI, 2, 2]),
                            op=AL.mult)

                # chunked ph2 (A) / ph3 (B); global interleave set by driver
                for qc_ in range(4):
                    emit_p2_chunk(qc_)
                    yield
                for qc_ in range(4):
                    emit_p3_chunk(qc_)
                    yield
                # ---------- Phase 4: channel stats (w-chunked big-op trees) -----
                # two w-chunks: [0:192] unlocks after B2, [192:256] after B3;
                # sum tree reduces in st, max tree in tree2, both over p1big
                st = big.tile([P, FH // 2], bf16, tag="tree", bufs=1)
                t2b = big.tile([P, FH // 2], bf16, tag="tree2", bufs=1)
                s_raw = sm.tile([P, FPC], bf16, tag="s_raw", bufs=1)
                mx = sm.tile([P, FPC], bf16, tag="mx", bufs=1)

                def tree_chunk(dst, op, target, w0, w1):
                    pv = p1big.rearrange("p (q pr c w) -> p q pr c w", q=4, pr=2, c=CT)
                    dv = dst.rearrange("p (q c w) -> p q c w", q=4, c=CT)
                    nc.vector.tensor_tensor(out=dv[:, :, :, w0:w1], in0=pv[:, :, 0, :, w0:w1],
                                            in1=pv[:, :, 1, :, w0:w1], op=op)
                    dq = dst.rearrange("p (qq pr c w) -> p qq pr c w", qq=2, pr=2, c=CT)
                    nc.vector.tensor_tensor(out=dq[:, 0, 0, :, w0:w1], in0=dq[:, 0, 0, :, w0:w1],
                                            in1=dq[:, 0, 1, :, w0:w1], op=op)
                    nc.vector.tensor_tensor(out=dq[:, 0, 1, :, w0:w1], in0=dq[:, 1, 0, :, w0:w1],
                                            in1=dq[:, 1, 1, :, w0:w1], op=op)
                    nc.vector.tensor_tensor(out=dv[:, 0, :, w0:w1], in0=dv[:, 0, :, w0:w1],
                                            in1=dv[:, 1, :, w0:w1], op=op)
                    cv = dv[:, 0]  # [p, c8, w]
                    nc.vector.tensor_tensor(out=cv[:, 0:4, w0:w1], in0=cv[:, 0:4, w0:w1],
                                            in1=cv[:, 4:8, w0:w1], op=op)
                    nc.vector.tensor_tensor(out=cv[:, 0:2, w0:w1], in0=cv[:, 0:2, w0:w1],
                                            in1=cv[:, 2:4, w0:w1], op=op)
                    nc.vector.tensor_tensor(out=target[:, w0:w1], in0=cv[:, 0, w0:w1],
                                            in1=cv[:, 1, w0:w1], op=op)

                _wc = FPC // 4
                for q_ in range(4):
                    tree_chunk(st, AL.add, s_raw, q_ * _wc, (q_ + 1) * _wc)
                    tree_chunk(t2b, AL.max, mx, q_ * _wc, (q_ + 1) * _wc)

                yield
                yield
                # g1 = sigmoid(cw0 * s_raw/64 + cw1 * mx + cb)
                t1 = sm.tile([P, FPC], bf16, tag="t1", bufs=1)
                nc.vector.tensor_scalar(out=t1, in0=s_raw, scalar1=cws[:, 0:1], scalar2=1.0 / C,
                                        op0=AL.mult, op1=AL.mult)
                t2 = sm.tile([P, FPC], bf16, tag="t2", bufs=1)
                nc.vector.tensor_scalar_mul(out=t2, in0=mx, scalar1=cws[:, 1:2])
                nc.vector.tensor_tensor(out=t1, in0=t1, in1=t2, op=AL.add)
                g1 = sm.tile([P, FPC], bf16, tag="g1", bufs=1)
                nc.scalar.activation(out=g1, in_=t1, func=AF.Sigmoid, bias=cws[:, 2:3], scale=1.0)

                # per-patch partial stats over b, both stats in one pipeline:
                # uc = [g1*s_raw | g1*mx] -> one b-reduce -> one transpose ->
                # one evac -> a-folds on disjoint partition halves
                uc = sm.tile([P, 2, FPC], bf16, tag="t2", bufs=1)
                nc.vector.tensor_tensor(out=uc[:, 0, :], in0=g1, in1=s_raw, op=AL.mult)
                nc.vector.tensor_tensor(out=uc[:, 1, :], in0=g1, in1=mx, op=AL.mult)
                pr_c = sm.tile([P, 2 * WI], bf16, tag="prmn", bufs=1)
                with nc.allow_low_precision(reason="4-term patch sum feeding a sigmoid gate"):
                    nc.vector.tensor_reduce(out=pr_c[:, 0:WI].rearrange("p (z wi) -> p z wi", z=1),
                                            in_=uc[:, 0:1, :].rearrange("p s (wi b) -> p s wi b", b=S),
                                            axis=mybir.AxisListType.X, op=AL.add)
                nc.vector.tensor_reduce(out=pr_c[:, WI:2 * WI].rearrange("p (z wi) -> p z wi", z=1),
                                        in_=uc[:, 1:2, :].rearrange("p s (wi b) -> p s wi b", b=S),
                                        axis=mybir.AxisListType.X, op=AL.max)
                pt = ps.tile([2 * WI, P], bf16, tag="pt", bufs=1)
                nc.tensor.transpose(pt, pr_c, ident)
                pte = sm.tile([2 * WI, P], bf16, tag="mne", bufs=1)
                nc.scalar.copy(out=pte, in_=pt)
                ptv = pte.rearrange("q (hi a) -> q hi a", a=S)
                fa = sm.tile([2 * WI, HIV, 2], bf16, tag="mnf", bufs=1)
                nc.vector.tensor_tensor(out=fa[0:WI], in0=ptv[0:WI, :, 0:2], in1=ptv[0:WI, :, 2:4], op=AL.add)
                nc.vector.tensor_tensor(out=fa[WI:2 * WI], in0=ptv[WI:2 * WI, :, 0:2],
                                        in1=ptv[WI:2 * WI, :, 2:4], op=AL.max)
                fo = sm.tile([2 * WI, HIV], bf16, tag="mno", bufs=1)
                nc.vector.tensor_tensor(out=fo[0:WI], in0=fa[0:WI, :, 0], in1=fa[0:WI, :, 1], op=AL.add)
                nc.vector.tensor_tensor(out=fo[WI:2 * WI], in0=fa[WI:2 * WI, :, 0],
                                        in1=fa[WI:2 * WI, :, 1], op=AL.max)
                mnT, mxT = fo[0:WI], fo[WI:2 * WI]

                # g2 = sigmoid(c2w0*mn/1024 + c2w1*mx + c2b) on [wi, hi]
                tg2 = sm.tile([WI, HIV], bf16, tag="tg2", bufs=1)
                nc.vector.tensor_scalar_mul(out=tg2, in0=mxT, scalar1=cws[0:WI, 4:5])
                tg = sm.tile([WI, HIV], bf16, tag="tg", bufs=1)
                nc.vector.scalar_tensor_tensor(out=tg, in0=mnT, scalar=cws[0:WI, 3:4],
                                               in1=tg2, op0=AL.mult, op1=AL.add)
                g2t2 = sm.tile([WI, P], bf16, tag="g2t2", bufs=1)
                nc.scalar.activation(out=g2t2.rearrange("q (hi a) -> q hi a", a=S),
                                     in_=tg.unsqueeze(2).broadcast_to([WI, HIV, S]),
                                     func=AF.Sigmoid, bias=cws[0:WI, 5:6], scale=1.0)
                pg = ps.tile([P, WI], bf16, tag="pg", bufs=1)
                nc.tensor.transpose(pg, g2t2, ident[0:WI, 0:WI])
                g2d = sm.tile([P, WI], bf16, tag="g2d", bufs=1)
                nc.vector.tensor_copy(out=g2d, in_=pg)

                # G = g1 * g2 (bf16, per pixel of this half)
                G = sm.tile([P, FPC], bf16, tag="G", bufs=1)
                nc.vector.tensor_tensor(
                    out=G.rearrange("p (wi b) -> p wi b", b=S),
                    in0=g1.rearrange("p (wi b) -> p wi b", b=S),
                    in1=g2d.unsqueeze(2).broadcast_to([P, WI, S]),
                    op=AL.mult)

                yield
                # ---------- Phase 5: out = p1 * G, store ------------------------
                for i5 in range(NT):
                    ot = big.tile([P, TF], bf16, tag=f"xb{v}", bufs=NT)
                    eng = nc.gpsimd if i5 in ((0, 1, 4, 6) if v == 0 else (1,)) else nc.vector
                    eng.tensor_tensor(
                        out=ot.rearrange("p (c w) -> p c w", c=CT),
                        in0=p1big[:, i5 * TF:(i5 + 1) * TF].rearrange("p (c w) -> p c w", c=CT),
                        in1=G.unsqueeze(1).broadcast_to([P, CT, FPC]),
                        op=AL.mult)
                    nc.sync.dma_start(out=y_v[v, :, i5 * CT:(i5 + 1) * CT, :],
                                      in_=ot.rearrange("p (c w) -> p c w", c=CT))

            gens = [emit_half(v) for v in range(NV)]
            next(gens[0], None)           # x loads v0
            emit_consts()                 # consts right behind v0's loads
            next(gens[1], None)           # x loads v1
            for g_ in gens:               # ph1 v0, ph1 v1
                next(g_, None)
            # ph2-all(v0), ph2-all(v1), ph3-all(v0), ph3-all(v1), tails
            # (the Tile scheduler is dependency-driven; this order only sets
            # tile-ring allocation order)
            for gi in [0] * 4 + [1] * 4 + [0] * 4 + [1] * 4 + [0] * 4 + [1] * 4:
                next(gens[gi], None)

    nc.compile()
    return nc


def _get_nc():
    if "nc" not in _CACHE:
        _CACHE["nc"] = _build()
    return _CACHE["nc"]


def kernel(x, fc_w, fc_b, conv1_w, conv1_b, conv2_w, conv2_b, size, **run_kwargs):
    from concourse.bass_utils import run_bass_kernel_spmd

    assert int(size) == S
    fcwT = np.ascontiguousarray(np.asarray(fc_w, dtype=np.float32).T)
    fcb = np.asarray(fc_b, dtype=np.float32)
    cws = np.concatenate([
        np.asarray(conv1_w, np.float32).ravel(), np.asarray(conv1_b, np.float32).ravel(),
        np.asarray(conv2_w, np.float32).ravel(), np.asarray(conv2_b, np.float32).ravel(),
    ]).astype(np.float32)
    assert cws.shape == (6,)
    cws[3] /= C * S * S          # mean-gate weight pre-scaled (STT has no scale)
    emat = np.zeros((C, 2 * C), np.float32)
    for c in range(C):
        emat[c, 2 * c:2 * c + 2] = 1.0

    import ml_dtypes
    x = np.ascontiguousarray(np.asarray(x).astype(ml_dtypes.bfloat16))
    fcwT = fcwT.astype(ml_dtypes.bfloat16)
    emat = emat.astype(ml_dtypes.bfloat16)

    nc = _get_nc()
    in_maps = [dict(x=x[i], fcwT=fcwT, fcb=fcb, cws=cws, emat=emat) for i in range(B)]
    res = run_bass_kernel_spmd(nc, in_maps, core_ids=list(range(B)), **run_kwargs)
    y = np.stack([res.results[i]["y"] for i in range(B)]).astype(np.float32)
    if run_kwargs:
        _CACHE["last_results"] = res
    return y



# revision 89
# speedup vs baseline: 1.0015x; 1.0015x over previous
"""Trainium2 Bass kernel for nn_CIAM patch-attention module.

Shapes (hardcoded): x [8, 64, 256, 256] f32, size=4.
Sharding: pure data parallel - one sample per NeuronCore (8 cores).

Per-core structure: the image is split into TOP/BOTTOM halves (128 rows each)
processed as two pipelined passes. Partition p = image row, free = c*256 + w
(w = wi*4 + b). x is pre-cast to bf16 on host and y is stored bf16 (cast back
on host), so all DMAs are plain HWDGE (no SWDGE descriptor-gen on Pool; the
cost model charges out-side bytes, halving store traffic). Phase 2 (64x64 FC
attention) runs in 2-group chunks: PE transposes + ACT evacs/sigmoid; m_e is
materialized with only 2 physical b-copies - phase 3 reads each value 4x via
a [stride-0,2][stride-1,2] innermost AP pair - which halves the PSUM->SBUF
evacuation and the b-expand matmul width. Phase 3 (p1 = x*m) is quarter-
chunked and split DVE/Pool. Phase 4's channel sum/max trees are w-quarter-
chunked big-view ops over a single p1 tile per half, so each quarter's tree
fires as soon as its phase-3 quarter lands (sum tree in one buffer, max tree
in another - they never alias). Per-patch gates fold via one merged
transpose; the whole gate chain is bf16. Phase 5 (y = p1*G) splits DVE/Pool
and streams stores per 8-channel tile. Pool only ever runs mult/copy ops
(TensorTensor max is not legal on Pool in neuronxcc).
"""
import sys
sys.path.insert(0, "/opt/trn_rl_repo")
import numpy as np

_CACHE = {}

B, C, H, W = 8, 64, 256, 256
S = 4
P = 128                # partitions = rows of one half-image
NV = 2                 # image halves (top/bottom)
HIV = P // S           # 32 patch rows per half
WI = W // S            # 64 patch cols
FPC = W                # free elems per channel (one row)
FH = C * FPC           # 16384 free elems per partition per half
CT = 8                 # channels per load tile
NT = C // CT           # 8 tiles
TF = CT * FPC          # 2048 free elems per (half, tile)


def _build():
    import concourse.bass as bass
    import concourse.bacc as bacc
    import concourse.tile as tile
    from concourse import mybir
    from concourse.masks import make_identity

    f32 = mybir.dt.float32
    bf16 = mybir.dt.bfloat16
    AL = mybir.AluOpType
    AF = mybir.ActivationFunctionType

    nc = bacc.Bacc("TRN2", target_bir_lowering=False, debug=False, num_devices=8)

    x_d = nc.dram_tensor("x", [C, H, W], bf16, kind="ExternalInput")
    fcwT_d = nc.dram_tensor("fcwT", [C, C], bf16, kind="ExternalInput")
    fcb_d = nc.dram_tensor("fcb", [C], f32, kind="ExternalInput")
    cws_d = nc.dram_tensor("cws", [6], f32, kind="ExternalInput")
    emat_d = nc.dram_tensor("emat", [C, 2 * C], bf16, kind="ExternalInput")
    y_d = nc.dram_tensor("y", [C, H, W], bf16, kind="ExternalOutput")

    # DRAM views: [half, row-in-half, c, w]
    x_v = x_d[:].rearrange("c (v r) w -> v r c w", v=NV)
    y_v = y_d[:].rearrange("c (v r) w -> v r c w", v=NV)

    with tile.TileContext(nc) as tc:
        with tc.tile_pool(name="big", bufs=1) as big, \
             tc.tile_pool(name="med", bufs=2) as med, \
             tc.tile_pool(name="sm", bufs=2) as sm, \
             tc.tile_pool(name="consts", bufs=1) as consts, \
             tc.tile_pool(name="ps", bufs=1, space="PSUM") as ps:

            # ---- constants (tiles only; DMAs emitted after the x loads) ----
            fcw = consts.tile([C, C], bf16)
            fcb = consts.tile([C, 1], f32)
            cws = consts.tile([P, 6], f32)
            emat = consts.tile([C, 2 * C], bf16)
            ident = consts.tile([P, P], bf16)
            identf = consts.tile([P, P], f32)

            def emit_consts():
                nc.sync.dma_start(out=fcw, in_=fcwT_d[:])         # pre-cast bf16, HWDGE
                nc.sync.dma_start(out=fcb, in_=fcb_d[:].unsqueeze(1))
                nc.sync.dma_start(out=cws, in_=bass.AP(tensor=cws_d, offset=0, ap=[[0, P], [1, 6]]))
                nc.sync.dma_start(out=emat, in_=emat_d[:])
                make_identity(nc, ident)
                make_identity(nc, identf)

            def emit_half(v):
                # loads first so HWDGE starts streaming x before anything else
                xbs = []   # (tile, first-ct, n-ct)
                sizes = [1] * NT
                ct0 = 0
                for nct in sizes:
                    xt = big.tile([P, nct * TF], bf16, tag=f"xb{v}", bufs=NT)
                    xbs.append((xt, ct0, nct))
                    nc.sync.dma_start(out=xt.rearrange("p (c w) -> p c w", c=nct * CT),
                                      in_=x_v[v, :, ct0 * CT:(ct0 + nct) * CT, :])
                    ct0 += nct

                yield
                # ---------- Phase 1: max over b (in-row patch pixels) -----------
                chmaxB = med.tile([P, C * WI], bf16, tag="chmax", bufs=2)  # wi-major: wi*64+c
                for ti, (xt, ct0, nct) in enumerate(xbs):
                    eng1 = nc.vector
                    for s_ in range(nct):
                        ct = ct0 + s_
                        v4 = xt[:, s_ * TF:(s_ + 1) * TF].rearrange("p (r pr u) -> p r pr u", pr=2, u=2)
                        r1 = sm.tile([P, CT * WI, 2], bf16, tag="r1", bufs=2)
                        eng1.tensor_tensor(out=r1, in0=v4[:, :, 0, :], in1=v4[:, :, 1, :], op=AL.max)
                        outv = chmaxB.rearrange("p (wi c) -> p c wi", c=C)[:, ct * CT:(ct + 1) * CT, :]
                        eng1.tensor_tensor(out=outv, in0=r1[:, :, 0], in1=r1[:, :, 1], op=AL.max)

                yield
                # ---------- Phase 2: FC attention -> m_e ------------------------
                # per group of 8 wi: build rhs [c, 8*32], one fc matmul (N=256),
                # one batched sigmoid (+a-dup), 8 transpose+b-expand matmuls with
                # the constant E matrix, one batched evacuation into m_e.
                # m_e as 4 wi-quarter tiles [c, wl(16), b] so P3 can start per quarter
                # m_e stores each (c, wi) gate value twice (t=2); phase 3 reads
                # it 4x via a [stride-0, 2][stride-1, 2] innermost AP pair
                m_eqs = []
                for q_ in range(4):
                    m_eq = med.tile([P, C * W // 8], bf16, tag="me", bufs=4)
                    m_eqs.append(m_eq)
                p1big = big.tile([P, FH], bf16, tag="p1", bufs=2)

                GW = 8                       # wi per group
                def emit_p2_chunk(qc):
                  for g in (2 * qc, 2 * qc + 1):
                    # 4 transposed chmax slices into one psum tile, one evac,
                    # one batched a-fold, two fold+scatter ops -> rhs_w
                    pa4 = ps.tile([P, 4 * P], bf16, tag="pa", bufs=2)
                    for j2 in range(4):
                        j = g * 4 + j2
                        nc.tensor.transpose(pa4[:, j2 * P:(j2 + 1) * P],
                                            chmaxB[:, j * P:(j + 1) * P], ident)
                    pae4 = sm.tile([P, 4 * P], bf16, tag="pae", bufs=1)
                    nc.scalar.copy(out=pae4, in_=pa4)
                    pav = pae4.rearrange("q (jj hi a) -> q (jj hi) a", jj=4, a=S)
                    f1 = sm.tile([P, 4 * HIV, 2], bf16, tag="f1", bufs=1)
                    nc.vector.tensor_tensor(out=f1, in0=pav[:, :, 0:2], in1=pav[:, :, 2:4], op=AL.max)
                    rhs_w = sm.tile([C, GW * HIV], bf16, tag="rhs_w", bufs=2)
                    rhs_b = rhs_w.rearrange("c (blk hi) -> c blk hi", hi=HIV)
                    for k in range(2):
                        # block index (2*jj + k) maps to wi = g*8 + block
                        nc.vector.tensor_tensor(
                            out=rhs_b[:, k:GW:2, :],
                            in0=f1[k * C:(k + 1) * C, :, 0].rearrange("c (jj hi) -> c jj hi", jj=4),
                            in1=f1[k * C:(k + 1) * C, :, 1].rearrange("c (jj hi) -> c jj hi", jj=4),
                            op=AL.max)
                    pmw = ps.tile([C, GW * HIV], f32, tag="pmw", bufs=2)
                    nc.tensor.matmul(pmw, fcw, rhs_w, start=True, stop=True)
                    # sigmoid + duplicate each hi column over the 4 patch rows
                    s2w = sm.tile([C, GW * P], bf16, tag="s2w", bufs=1)
                    nc.scalar.activation(
                        out=s2w.rearrange("c (wl hi a) -> c wl hi a", wl=GW, a=S),
                        in_=pmw.rearrange("c (wl hi) -> c wl hi", wl=GW).unsqueeze(3).broadcast_to([C, GW, HIV, S]),
                        func=AF.Sigmoid, bias=fcb, scale=1.0)
                    for sg in range(2):
                        pe4 = ps.tile([P, GW // 2 * C * 2], f32, tag="pe4", bufs=1)
                        for wl2 in range(GW // 2):
                            wl = sg * (GW // 2) + wl2
                            nc.tensor.matmul(pe4[:, wl2 * C * 2:(wl2 + 1) * C * 2],
                                             s2w[:, wl * P:(wl + 1) * P],
                                             emat, start=True, stop=True)
                        # batched evacuation: psum [(wl c t)] -> m_eq (c, wi, t)
                        w0l = (g % 2) * GW + sg * (GW // 2)
                        me_v = m_eqs[g // 2].rearrange("p (c wi t) -> p wi c t", c=C, t=2)[:, w0l:w0l + GW // 2, :, :]
                        nc.scalar.copy(out=me_v, in_=pe4.rearrange("p (wl c t) -> p wl c t", wl=GW // 2, t=2))

                def emit_p3_chunk(q_):
                    # phase 3 for quarter q_: p1 = x * m over all ct tiles;
                    # in1 reads each m value 4x via [0-stride,2][1-stride,2]
                    WQ = W // 4
                    NWI = WQ // S
                    for t3, (xt, ct0, nct) in enumerate(xbs):
                        p1t = p1big[:, t3 * TF:(t3 + 1) * TF]
                        ncc = nct * CT
                        eng3 = nc.gpsimd if t3 in (2, 5, 6) else nc.vector
                        eng3.tensor_tensor(
                            out=p1t.rearrange("p (c wi d t) -> p c wi d t", c=ncc, d=2, t=2)[:, :, q_ * NWI:(q_ + 1) * NWI, :, :],
                            in0=xt.rearrange("p (c wi d t) -> p c wi d t", c=ncc, d=2, t=2)[:, :, q_ * NWI:(q_ + 1) * NWI, :, :],
                            in1=m_eqs[q_].rearrange("p (c wi t) -> p c wi t", c=C, t=2)[:, ct0 * CT:(ct0 + nct) * CT, :, :]
                                .unsqueeze(3).broadcast_to([P, ncc, NWI, 2, 2]),
                            op=AL.mult)

                # chunked ph2 (A) / ph3 (B); global interleave set by driver
                for qc_ in range(4):
                    emit_p2_chunk(qc_)
                    yield
                for qc_ in range(4):
                    emit_p3_chunk(qc_)
                    yield
                # ---------- Phase 4: channel stats (w-chunked big-op trees) -----
                # two w-chunks: [0:192] unlocks after B2, [192:256] after B3;
                # sum tree reduces in st, max tree in tree2, both over p1big
                st = big.tile([P, FH // 2], bf16, tag="tree", bufs=1)
                t2b = big.tile([P, FH // 2], bf16, tag="tree2", bufs=1)
                s_raw = sm.tile([P, FPC], bf16, tag="s_raw", bufs=1)
                mx = sm.tile([P, FPC], bf16, tag="mx", bufs=1)

                def tree_chunk(dst, op, target, w0, w1):
                    pv = p1big.rearrange("p (q pr c w) -> p q pr c w", q=4, pr=2, c=CT)
                    dv = dst.rearrange("p (q c w) -> p q c w", q=4, c=CT)
                    nc.vector.tensor_tensor(out=dv[:, :, :, w0:w1], in0=pv[:, :, 0, :, w0:w1],
                                            in1=pv[:, :, 1, :, w0:w1], op=op)
                    dq = dst.rearrange("p (qq pr c w) -> p qq pr c w", qq=2, pr=2, c=CT)
                    nc.vector.tensor_tensor(out=dq[:, 0, 0, :, w0:w1], in0=dq[:, 0, 0, :, w0:w1],
                                            in1=dq[:, 0, 1, :, w0:w1], op=op)
                    nc.vector.tensor_tensor(out=dq[:, 0, 1, :, w0:w1], in0=dq[:, 1, 0, :, w0:w1],
                                            in1=dq[:, 1, 1, :, w0:w1], op=op)
                    nc.vector.tensor_tensor(out=dv[:, 0, :, w0:w1], in0=dv[:, 0, :, w0:w1],
                                            in1=dv[:, 1, :, w0:w1], op=op)
                    cv = dv[:, 0]  # [p, c8, w]
                    nc.vector.tensor_tensor(out=cv[:, 0:4, w0:w1], in0=cv[:, 0:4, w0:w1],
                                            in1=cv[:, 4:8, w0:w1], op=op)
                    nc.vector.tensor_tensor(out=cv[:, 0:2, w0:w1], in0=cv[:, 0:2, w0:w1],
                                            in1=cv[:, 2:4, w0:w1], op=op)
                    nc.vector.tensor_tensor(out=target[:, w0:w1], in0=cv[:, 0, w0:w1],
                                            in1=cv[:, 1, w0:w1], op=op)

                _wc = FPC // 4
                for q_ in range(4):
                    tree_chunk(st, AL.add, s_raw, q_ * _wc, (q_ + 1) * _wc)
                    tree_chunk(t2b, AL.max, mx, q_ * _wc, (q_ + 1) * _wc)

                yield
                yield
                # g1 = sigmoid(cw0 * s_raw/64 + cw1 * mx + cb)
                t1 = sm.tile([P, FPC], bf16, tag="t1", bufs=1)
                nc.vector.tensor_scalar(out=t1, in0=s_raw, scalar1=cws[:, 0:1], scalar2=1.0 / C,
                                        op0=AL.mult, op1=AL.mult)
                t2 = sm.tile([P, FPC], bf16, tag="t2", bufs=1)
                nc.vector.tensor_scalar_mul(out=t2, in0=mx, scalar1=cws[:, 1:2])
                nc.vector.tensor_tensor(out=t1, in0=t1, in1=t2, op=AL.add)
                g1 = sm.tile([P, FPC], bf16, tag="g1", bufs=1)
                nc.scalar.activation(out=g1, in_=t1, func=AF.Sigmoid, bias=cws[:, 2:3], scale=1.0)

                # per-patch partial stats over b, both stats in one pipeline:
                # uc = [g1*s_raw | g1*mx] -> one b-reduce -> one transpose ->
                # one evac -> a-folds on disjoint partition halves
                uc = sm.tile([P, 2, FPC], bf16, tag="t2", bufs=1)
                nc.vector.tensor_tensor(out=uc[:, 0, :], in0=g1, in1=s_raw, op=AL.mult)
                nc.vector.tensor_tensor(out=uc[:, 1, :], in0=g1, in1=mx, op=AL.mult)
                pr_c = sm.tile([P, 2 * WI], bf16, tag="prmn", bufs=1)
                with nc.allow_low_precision(reason="4-term patch sum feeding a sigmoid gate"):
                    nc.vector.tensor_reduce(out=pr_c[:, 0:WI].rearrange("p (z wi) -> p z wi", z=1),
                                            in_=uc[:, 0:1, :].rearrange("p s (wi b) -> p s wi b", b=S),
                                            axis=mybir.AxisListType.X, op=AL.add)
                nc.vector.tensor_reduce(out=pr_c[:, WI:2 * WI].rearrange("p (z wi) -> p z wi", z=1),
                                        in_=uc[:, 1:2, :].rearrange("p s (wi b) -> p s wi b", b=S),
                                        axis=mybir.AxisListType.X, op=AL.max)
                pt = ps.tile([2 * WI, P], bf16, tag="pt", bufs=1)
                nc.tensor.transpose(pt, pr_c, ident)
                pte = sm.tile([2 * WI, P], bf16, tag="mne", bufs=1)
                nc.scalar.copy(out=pte, in_=pt)
                ptv = pte.rearrange("q (hi a) -> q hi a", a=S)
                fa = sm.tile([2 * WI, HIV, 2], bf16, tag="mnf", bufs=1)
                nc.vector.tensor_tensor(out=fa[0:WI], in0=ptv[0:WI, :, 0:2], in1=ptv[0:WI, :, 2:4], op=AL.add)
                nc.vector.tensor_tensor(out=fa[WI:2 * WI], in0=ptv[WI:2 * WI, :, 0:2],
                                        in1=ptv[WI:2 * WI, :, 2:4], op=AL.max)
                fo = sm.tile([2 * WI, HIV], bf16, tag="mno", bufs=1)
                nc.vector.tensor_tensor(out=fo[0:WI], in0=fa[0:WI, :, 0], in1=fa[0:WI, :, 1], op=AL.add)
                nc.vector.tensor_tensor(out=fo[WI:2 * WI], in0=fa[WI:2 * WI, :, 0],
                                        in1=fa[WI:2 * WI, :, 1], op=AL.max)
                mnT, mxT = fo[0:WI], fo[WI:2 * WI]

                # g2 = sigmoid(c2w0*mn/1024 + c2w1*mx + c2b) on [wi, hi]
                tg2 = sm.tile([WI, HIV], bf16, tag="tg2", bufs=1)
                nc.vector.tensor_scalar_mul(out=tg2, in0=mxT, scalar1=cws[0:WI, 4:5])
                tg = sm.tile([WI, HIV], bf16, tag="tg", bufs=1)
                nc.vector.scalar_tensor_tensor(out=tg, in0=mnT, scalar=cws[0:WI, 3:4],
                                               in1=tg2, op0=AL.mult, op1=AL.add)
                g2t2 = sm.tile([WI, P], bf16, tag="g2t2", bufs=1)
                nc.scalar.activation(out=g2t2.rearrange("q (hi a) -> q hi a", a=S),
                                     in_=tg.unsqueeze(2).broadcast_to([WI, HIV, S]),
                                     func=AF.Sigmoid, bias=cws[0:WI, 5:6], scale=1.0)
                pg = ps.tile([P, WI], bf16, tag="pg", bufs=1)
                nc.tensor.transpose(pg, g2t2, ident[0:WI, 0:WI])
                # G = g1 * g2 (bf16, per pixel of this half); g2 read straight
                # from the PSUM transpose (op is 1x either way)
                G = sm.tile([P, FPC], bf16, tag="G", bufs=1)
                nc.vector.tensor_tensor(
                    out=G.rearrange("p (wi b) -> p wi b", b=S),
                    in0=g1.rearrange("p (wi b) -> p wi b", b=S),
                    in1=pg.unsqueeze(2).broadcast_to([P, WI, S]),
                    op=AL.mult)

                yield
                # ---------- Phase 5: out = p1 * G, store ------------------------
                for i5 in range(NT):
                    ot = big.tile([P, TF], bf16, tag=f"xb{v}", bufs=NT)
                    eng = nc.gpsimd if i5 in ((0, 1, 4, 6) if v == 0 else (1,)) else nc.vector
                    eng.tensor_tensor(
                        out=ot.rearrange("p (c w) -> p c w", c=CT),
                        in0=p1big[:, i5 * TF:(i5 + 1) * TF].rearrange("p (c w) -> p c w", c=CT),
                        in1=G.unsqueeze(1).broadcast_to([P, CT, FPC]),
                        op=AL.mult)
                    nc.sync.dma_start(out=y_v[v, :, i5 * CT:(i5 + 1) * CT, :],
                                      in_=ot.rearrange("p (c w) -> p c w", c=CT))

            gens = [emit_half(v) for v in range(NV)]
            next(gens[0], None)           # x loads v0
            emit_consts()                 # consts right behind v0's loads
            next(gens[1], None)           # x loads v1
            for g_ in gens:               # ph1 v0, ph1 v1
                next(g_, None)
            # ph2-all(v0), ph2-all(v1), ph3-all(v0), ph3-all(v1), tails
            # (the Tile scheduler is dependency-driven; this order only sets
            # tile-ring allocation order)
            for gi in [0] * 4 + [1] * 4 + [0] * 4 + [1] * 4 + [0] * 4 + [1] * 4:
                next(gens[gi], None)

    nc.compile()
    return nc


def _get_nc():
    if "nc" not in _CACHE:
        _CACHE["nc"] = _build()
    return _CACHE["nc"]


def kernel(x, fc_w, fc_b, conv1_w, conv1_b, conv2_w, conv2_b, size, **run_kwargs):
    from concourse.bass_utils import run_bass_kernel_spmd

    assert int(size) == S
    fcwT = np.ascontiguousarray(np.asarray(fc_w, dtype=np.float32).T)
    fcb = np.asarray(fc_b, dtype=np.float32)
    cws = np.concatenate([
        np.asarray(conv1_w, np.float32).ravel(), np.asarray(conv1_b, np.float32).ravel(),
        np.asarray(conv2_w, np.float32).ravel(), np.asarray(conv2_b, np.float32).ravel(),
    ]).astype(np.float32)
    assert cws.shape == (6,)
    cws[3] /= C * S * S          # mean-gate weight pre-scaled (STT has no scale)
    emat = np.zeros((C, 2 * C), np.float32)
    for c in range(C):
        emat[c, 2 * c:2 * c + 2] = 1.0

    import ml_dtypes
    x = np.ascontiguousarray(np.asarray(x).astype(ml_dtypes.bfloat16))
    fcwT = fcwT.astype(ml_dtypes.bfloat16)
    emat = emat.astype(ml_dtypes.bfloat16)

    nc = _get_nc()
    in_maps = [dict(x=x[i], fcwT=fcwT, fcb=fcb, cws=cws, emat=emat) for i in range(B)]
    res = run_bass_kernel_spmd(nc, in_maps, core_ids=list(range(B)), **run_kwargs)
    y = np.stack([res.results[i]["y"] for i in range(B)]).astype(np.float32)
    if run_kwargs:
        _CACHE["last_results"] = res
    return y

